# revision 25
# baseline (speedup 1.0000x reference)
"""GQA attention (B=2, S=2048, D=1024, H=16, KV=4, HD=64) with RoPE + causal
softmax + output projection, sharded over 8 trn2 NeuronCores.

v2 design (per core: one batch x one KV group = 4 q heads + 1 kv head):
  - Q/K projections in fp8e4 DoubleRow (contract 1024 as 4 stacked pairs of
    128-row k-tiles, 2 cols/cycle).  Weights x64 on host; rope tables carry
    1/64 so qt/kt come out true-scale bf16.
  - V projection in bf16 (accuracy: early queries see V almost directly).
  - scores^T row-packed: two 64-contract matmuls on PE row halves 0:63/64:127
    run concurrently (kt/qt duplicated to partitions 64:128 via SBUF DMA).
  - exp on ACT with fused scale 1/sqrt(HD); one instr per chunk-pair, the
    above-diagonal strip is computed+exp'd then zeroed by the [0|tri] mask.
  - AV: span 0 (queries 0:512) in bf16; spans 1:4 in fp8e4 DoubleRow over
    chunk pairs (the 1/16 fp8 truncation bias is pre-compensated x1.03125;
    per-element fp8 noise averages out over >=512 keys).  The ones-columns
    in vaug give the softmax denominators for free.
  - out projection bf16; partial [S,D] outputs summed on host per batch.
  - interleaved schedule: QKV+rope span sp runs while attention span sp-1
    streams; elementwise work split across DVE and Pool.
"""

import sys

if "/opt/trn_rl_repo" not in sys.path:
    sys.path.insert(0, "/opt/trn_rl_repo")

import numpy as np
import ml_dtypes

B, S, D = 2, 2048, 1024
H, KV, HD = 16, 4, 64
NHC = H // KV          # query heads per core = 4
FQ = NHC * HD          # 256
FQK = FQ + HD          # 320 fp8 qk columns
SPAN = 512
NSPAN = S // SPAN      # 4
NCHUNK = S // 128      # 16
KD = D // 128          # 8
BF16 = ml_dtypes.bfloat16
FP8 = ml_dtypes.float8_e4m3

_CACHE = {}


def _build(debug_taps=False):
    key = ("nc", debug_taps)
    if key in _CACHE:
        return _CACHE[key]

    import concourse.bass as bass
    import concourse.tile as tile
    from concourse import bacc, mybir

    f32 = mybir.dt.float32
    bf16 = mybir.dt.bfloat16
    fp8 = mybir.dt.float8e4
    ADD = mybir.AluOpType.add
    MUL = mybir.AluOpType.mult
    EXP = mybir.ActivationFunctionType.Exp
    DR = mybir.MatmulPerfMode.DoubleRow
    ts = bass.ts

    nc = bacc.Bacc("TRN2", target_bir_lowering=False, debug=False)

    xt8_d = nc.dram_tensor("xt8", [D, S], fp8, kind="ExternalInput").ap()
    xt16_d = nc.dram_tensor("xt16", [D, S], bf16, kind="ExternalInput").ap()
    wqk_d = nc.dram_tensor("wqk", [D, FQK], fp8, kind="ExternalInput").ap()
    wv_d = nc.dram_tensor("wv", [D, HD], bf16, kind="ExternalInput").ap()
    wo_d = nc.dram_tensor("wo", [FQ, D], bf16, kind="ExternalInput").ap()
    cos2_d = nc.dram_tensor("cos2", [128, S], bf16, kind="ExternalInput").ap()
    sina2_d = nc.dram_tensor("sina2", [128, S], bf16, kind="ExternalInput").ap()
    mt8_d = nc.dram_tensor("mt8", [128, 128], fp8, kind="ExternalInput").ap()
    mt16_d = nc.dram_tensor("mt16", [128, 128], bf16, kind="ExternalInput").ap()
    mneg_d = nc.dram_tensor("mneg", [128, 128], bf16, kind="ExternalInput").ap()
    idb_d = nc.dram_tensor("idb", [128, 128], bf16, kind="ExternalInput").ap()
    idt16_d = nc.dram_tensor("idt16", [64, 64], bf16, kind="ExternalInput").ap()
    out_d = nc.dram_tensor("out", [S, D], bf16, kind="ExternalOutput").ap()

    if debug_taps:
        dbg_qt = nc.dram_tensor("dbg_qt", [128, NHC, S], bf16, kind="ExternalOutput").ap()
        dbg_kt = nc.dram_tensor("dbg_kt", [128, S], bf16, kind="ExternalOutput").ap()
        dbg_vaug8 = nc.dram_tensor("dbg_vaug8", [128, NCHUNK, 128], fp8, kind="ExternalOutput").ap()
        dbg_at = nc.dram_tensor("dbg_at", [128, 2, S], bf16, kind="ExternalOutput").ap()

    xt8_v = xt8_d.rearrange("(ko p) s -> p ko s", p=128)
    xt16_v = xt16_d.rearrange("(ko p) s -> p ko s", p=128)
    wqk_v = wqk_d.rearrange("(ko p) f -> p ko f", p=128)
    wv_v = wv_d.rearrange("(ko p) f -> p ko f", p=128)
    wo_v = wo_d.rearrange("(c p) n -> p c n", p=128)
    out_v = out_d.rearrange("(t p) n -> p t n", p=128)

    with tile.TileContext(nc) as tc:
        with tc.tile_pool(name="consts", bufs=1) as consts, \
             tc.tile_pool(name="persist", bufs=1) as persist, \
             tc.tile_pool(name="xtp", bufs=3) as xtp, \
             tc.tile_pool(name="ropet", bufs=4) as ropet, \
             tc.tile_pool(name="cpp", bufs=3) as cpp, \
             tc.tile_pool(name="vtp", bufs=2) as vtp, \
             tc.tile_pool(name="etp", bufs=6) as etp, \
             tc.tile_pool(name="lp", bufs=3) as lp, \
             tc.tile_pool(name="obp", bufs=4) as obp, \
             tc.tile_pool(name="psA", bufs=2, space="PSUM") as psA, \
             tc.tile_pool(name="psS", bufs=2, space="PSUM") as psS, \
             tc.tile_pool(name="psAV", bufs=1, space="PSUM") as psAV:

            wqk_sb = consts.tile([128, KD, FQK], fp8, name="wqk_sb")
            wv_sb = consts.tile([128, KD, HD], bf16, name="wv_sb")
            cos2_sb = consts.tile([128, S], bf16, name="cos2_sb")
            sina2_sb = consts.tile([128, S], bf16, name="sina2_sb")
            mt8_sb = consts.tile([128, 128], fp8, name="mt8_sb")
            mt16_sb = consts.tile([128, 128], bf16, name="mt16_sb")
            mneg_sb = consts.tile([128, 128], bf16, name="mneg_sb")
            idb_sb = consts.tile([128, 128], bf16, name="idb_sb")
            idt16_sb = consts.tile([64, 64], bf16, name="idt16_sb")
            wo_sb = consts.tile([128, 2, D], bf16, name="wo_sb")

            qt_sb = persist.tile([128, NHC, S], bf16, name="qt_sb")
            kt_sb = persist.tile([128, S], bf16, name="kt_sb")
            vaug8_sb = persist.tile([128, NCHUNK, 128], fp8, name="vaug8_sb")
            vaug16_sb = persist.tile([128, SPAN // 128, 128], bf16, name="vaug16_sb")
            at_sb = persist.tile([128, 2, S], bf16, name="at_sb")

            # minimal prefix for the first matmuls
            nc.sync.dma_start(wqk_sb[:], wqk_v[:])

            xts8_list, xts16_list = [], []
            for sp in range(NSPAN):
                rng = slice(sp * SPAN, (sp + 1) * SPAN)
                xts8 = xtp.tile([128, KD, SPAN], fp8, name="xts8", tag="x8")
                nc.sync.dma_start(xts8[:], xt8_v[:, :, rng])
                if sp == 0:
                    # rope tables + small mask/identity consts first: attn(0)
                    # diag pairs need them long before xts16 is consumed
                    nc.sync.dma_start(cos2_sb[:], cos2_d[:])
                    nc.sync.dma_start(sina2_sb[:], sina2_d[:])
                    nc.sync.dma_start(mneg_sb[:], mneg_d[:])
                    nc.sync.dma_start(idb_sb[:], idb_d[:])
                    nc.sync.dma_start(mt8_sb[:], mt8_d[:])
                    nc.sync.dma_start(mt16_sb[:], mt16_d[:])
                    nc.sync.dma_start(idt16_sb[:], idt16_d[:])
                    nc.sync.dma_start(wv_sb[:], wv_v[:])
                xts16 = xtp.tile([128, KD, SPAN], bf16, name="xts16", tag="x16")
                nc.sync.dma_start(xts16[:], xt16_v[:, :, rng])
                xts8_list.append(xts8)
                xts16_list.append(xts16)
                if sp == 0:
                    nc.gpsimd.memset(vaug8_sb[:, :, 0:64], 1.0)
                    nc.gpsimd.memset(vaug16_sb[:, :, 0:64], 1.0)
                if sp == 1:
                    nc.sync.dma_start(wo_sb[:], wo_v[:])

            def qkv_units(sp):
                """Closures: Q/K fp8-DR + rope per ftile, V proj, V transpose.
                Emitted interleaved between attention pairs so the PE stays
                dense and the ACT engine is fed continuously."""
                rng = slice(sp * SPAN, (sp + 1) * SPAN)
                xts8 = xts8_list[sp]
                xts16 = xts16_list[sp]
                vt16_cell = {}

                def ft_unit(ft):
                    def run():
                        fcols = 128 if ft < 2 else 64
                        ps = psA.tile([128, SPAN], f32, name="psqk", tag="pa")
                        for k in range(KD // 2):
                            nc.tensor.matmul(
                                ps[0:fcols, :],
                                lhsT=wqk_sb[:, 2 * k:2 * k + 2,
                                            ts(ft, 128) if ft < 2 else slice(FQ, FQK)],
                                rhs=xts8[:, 2 * k:2 * k + 2, :],
                                start=(k == 0),
                                stop=(k == KD // 2 - 1),
                                perf_mode=DR,
                            )
                        np_lo = fcols
                        # bounce psum to SBUF bf16 once; rope runs on DVE bf16
                        cp = cpp.tile([128, SPAN], bf16, name="cp", tag="cp")
                        nc.vector.tensor_copy(cp[0:np_lo, :], ps[0:np_lo])
                        t = ropet.tile([128, 2, SPAN], bf16, name="ropet", tag="rt")
                        nc.vector.tensor_tensor(
                            t[0:np_lo, 0, :], cp[0:np_lo], cos2_sb[0:np_lo, rng], op=MUL
                        )
                        for o in range(0, np_lo, 64):
                            nc.vector.tensor_tensor(
                                t[o:o + 32, 1, :], cp[o + 32:o + 64],
                                sina2_sb[o + 32:o + 64, rng], op=MUL,
                            )
                            nc.vector.tensor_tensor(
                                t[o + 32:o + 64, 1, :], cp[o:o + 32],
                                sina2_sb[o:o + 32, rng], op=MUL,
                            )
                        if ft < 2:
                            nc.vector.tensor_tensor(
                                qt_sb[0:64, 2 * ft, rng], t[0:64, 0, :], t[0:64, 1, :], op=ADD
                            )
                            nc.vector.tensor_tensor(
                                qt_sb[0:64, 2 * ft + 1, rng], t[64:128, 0, :], t[64:128, 1, :], op=ADD
                            )
                            # duplicate for row-packed scores as soon as ready
                            nc.sync.dma_start(
                                qt_sb[64:128, 2 * ft:2 * ft + 2, rng],
                                qt_sb[0:64, 2 * ft:2 * ft + 2, rng],
                            )
                        else:
                            nc.vector.tensor_tensor(
                                kt_sb[0:64, rng], t[0:64, 0, :], t[0:64, 1, :], op=ADD
                            )
                            nc.sync.dma_start(kt_sb[64:128, rng], kt_sb[0:64, rng])
                    return run

                def v_proj():
                    psv = psA.tile([128, SPAN], f32, name="psv", tag="pa")
                    for k in range(KD):
                        nc.tensor.matmul(
                            psv[0:64, :],
                            lhsT=wv_sb[:, k, :],
                            rhs=xts16[:, k, :],
                            start=(k == 0),
                            stop=(k == KD - 1),
                        )
                    vt16 = vtp.tile([64, SPAN], bf16, name="vt16", tag="vt16")
                    nc.vector.tensor_scalar_mul(vt16[:], psv[0:64, :], 16.0)
                    vt16_cell["vt"] = vt16

                def v_transp():
                    vt16 = vt16_cell["vt"]
                    tps16 = psA.tile([128, SPAN // 128, 64], bf16, name="tps16", tag="pa")
                    for c in range(SPAN // 128):
                        nc.tensor.transpose(tps16[:, c, :], vt16[:, ts(c, 128)], idt16_sb[:])
                    nc.vector.tensor_scalar_mul(
                        vaug8_sb[:, sp * 4:sp * 4 + 4, 64:128], tps16[:], 1.03125)
                    if sp == 0:
                        nc.vector.tensor_copy(vaug16_sb[:, 0:4, 64:128], tps16[:])

                return [ft_unit(2), ft_unit(0), ft_unit(1), v_proj, v_transp]

            def proj_units(s):
                """Out-projection closures for span s (at_sb already final)."""
                units = []
                for i, st in enumerate(range(4 * s, 4 * s + 4)):
                    for no in range(2):
                        def run(i=i, st=st, no=no):
                            pc = psA.tile([128, SPAN], f32, name="pc", tag="pa")
                            for c in range(2):
                                nc.tensor.matmul(
                                    pc[:],
                                    lhsT=at_sb[:, c, ts(st, 128)],
                                    rhs=wo_sb[:, c, ts(no, SPAN)],
                                    start=(c == 0),
                                    stop=(c == 1),
                                )
                            ob = obp.tile([128, SPAN], bf16, name="ob", tag="ob")
                            nc.vector.tensor_copy(ob[:], pc[:])
                            nc.sync.dma_start(out_v[:, st, ts(no, SPAN)], ob[:])
                        units.append(run)
                return units

            def attn_emitters(s):
                """Pair closures (scores+exp+masks, AV pipelined one behind)
                and per-pp norm closures; returns the flat emission list."""
                npair = 2 * s + 2
                use8 = s > 0
                ctx = {"pending": None, "avs": None}

                def emit_av(hi, t, et):
                    av = ctx["avs"][hi]
                    j0, j1 = 2 * t, 2 * t + 1
                    lo0 = max(j0 * 128 - s * SPAN, 0)
                    lo1 = max(j1 * 128 - s * SPAN, 0)
                    if use8:
                        nc.tensor.matmul(
                            av[:, lo0:SPAN],
                            lhsT=vaug8_sb[:, j0:j0 + 2, :],
                            rhs=et[:, 0:2, lo0:SPAN],
                            start=(t == 0),
                            stop=(t == npair - 1),
                            perf_mode=DR,
                        )
                    else:
                        for par, j, lo in ((0, j0, lo0), (1, j1, lo1)):
                            nc.tensor.matmul(
                                av[:, lo:SPAN],
                                lhsT=vaug16_sb[:, j, :],
                                rhs=et[:, par, lo:SPAN],
                                start=(j == 0),
                                stop=(j == npair * 2 - 1),
                            )

                def pair(pp, hi, t):
                    def run():
                        if hi == 0 and t == 0:
                            ctx["avs"] = [
                                psAV.tile([128, SPAN], f32, name="av", tag="av", bufs=2)
                                for _ in range(2)]
                        h = 2 * pp + hi
                        j0, j1 = 2 * t, 2 * t + 1
                        lo0 = max(j0 * 128 - s * SPAN, 0)
                        diag = j0 >= 4 * s
                        sc = psS.tile([128, 2, SPAN], f32, name="sc", tag="sc")
                        # row-packed pair: PE rows 0:63 / 64:127; j1 from lo0,
                        # the invalid strip gets -3000 added so exp -> 0
                        nc.tensor.matmul(
                            sc[:, 0, lo0:SPAN],
                            lhsT=kt_sb[0:64, ts(j0, 128)],
                            rhs=qt_sb[0:64, h, s * SPAN + lo0:(s + 1) * SPAN],
                            start=True, stop=True,
                        )
                        nc.tensor.matmul(
                            sc[:, 1, lo0:SPAN],
                            lhsT=kt_sb[64:128, ts(j1, 128)],
                            rhs=qt_sb[64:128, h, s * SPAN + lo0:(s + 1) * SPAN],
                            start=True, stop=not diag,
                        )
                        if diag:
                            nc.tensor.matmul(
                                sc[:, 1, lo0:lo0 + 128],
                                lhsT=mneg_sb[:],
                                rhs=idb_sb[:],
                                start=False, stop=True,
                            )
                        if ctx["pending"] is not None:
                            emit_av(*ctx["pending"])
                            ctx["pending"] = None
                        et = etp.tile([128, 2, SPAN], fp8 if use8 else bf16,
                                      name="et", tag="et8" if use8 else "et16")
                        nc.scalar.activation(
                            et[:, :, lo0:SPAN], sc[:, :, lo0:SPAN], EXP,
                            scale=0.125,
                        )
                        if diag:
                            nc.gpsimd.tensor_tensor(
                                et[:, 0, lo0:lo0 + 128],
                                et[:, 0, lo0:lo0 + 128],
                                mt8_sb[:] if use8 else mt16_sb[:], op=MUL,
                            )
                            nc.gpsimd.tensor_tensor(
                                et[:, 1, lo0 + 128:lo0 + 256],
                                et[:, 1, lo0 + 128:lo0 + 256],
                                mt8_sb[:] if use8 else mt16_sb[:], op=MUL,
                            )
                        ctx["pending"] = (hi, t, et)
                    return run

                def norm_pp(pp):
                    def run():
                        if ctx["pending"] is not None:
                            emit_av(*ctx["pending"])
                            ctx["pending"] = None
                        for hi in range(2):
                            av = ctx["avs"][hi]
                            linv = lp.tile([64, SPAN], f32, name="linv", tag="lv")
                            nc.vector.reciprocal_approx_fast(
                                out=linv[:], in_=av[0:64, :]
                            )
                            dst = at_sb[64 * hi:64 * (hi + 1), pp,
                                        s * SPAN:(s + 1) * SPAN]
                            nc.vector.tensor_tensor(dst, av[64:128, :], linv[:], op=MUL)
                    return run

                items = []
                for pp in range(2):
                    for hi in range(2):
                        for t in range(npair):
                            items.append(pair(pp, hi, t))
                    items.append(norm_pp(pp))
                return items

            def run_interleaved(items, units):
                n, m = len(items), len(units)
                ui = 0
                for idx, it in enumerate(items):
                    it()
                    while ui < m and (idx + 1) * (m + 1) >= (ui + 1) * n:
                        units[ui]()
                        ui += 1
                while ui < m:
                    units[ui]()
                    ui += 1

            # two spans of QKV up front: the PE stays dense while both
            # ropes drain on DVE, so attention(0) starts stall-free
            for u in qkv_units(0):
                u()
            for u in qkv_units(1):
                u()
            run_interleaved(attn_emitters(0), qkv_units(2))
            run_interleaved(attn_emitters(1), qkv_units(3) + proj_units(0))
            run_interleaved(attn_emitters(2), proj_units(1))
            run_interleaved(attn_emitters(3), proj_units(2))
            for u in proj_units(3):
                u()

            if debug_taps:
                nc.sync.dma_start(dbg_qt[:], qt_sb[:])
                nc.sync.dma_start(dbg_kt[:], kt_sb[:])
                nc.sync.dma_start(dbg_vaug8[:], vaug8_sb[:])
                nc.sync.dma_start(dbg_at[:], at_sb[:])

    nc.compile()
    _CACHE[key] = nc
    return nc


def _prep_inputs(x, cos, sin, Wq, Wk, Wv, Wo):
    """Build the 8 per-core input maps (host-side sharding + layout prep)."""
    x = np.asarray(x, np.float32)
    cos = np.asarray(cos, np.float32)
    sin = np.asarray(sin, np.float32)
    Wq = np.asarray(Wq, np.float32)
    Wk = np.asarray(Wk, np.float32)
    Wv = np.asarray(Wv, np.float32)
    Wo = np.asarray(Wo, np.float32)

    SW = 64.0

    cosT = cos.T.copy() * (1.0 / SW)          # [HD, S]
    sinT = sin.T.copy() * (1.0 / SW)
    cos2 = np.tile(cosT, (2, 1)).astype(BF16)                 # [128, S]
    # indexed by the INPUT partition of the rotate-half read (SBUF+SBUF
    # tensor_tensor requires equal input base partitions)
    sina = np.concatenate([sinT[32:64], -sinT[0:32]], axis=0)
    sina2 = np.tile(sina, (2, 1)).astype(BF16)                # [128, S]

    p = np.arange(128)[:, None]
    f = np.arange(128)[None, :]
    mt = np.where(p <= f, 1.0, 0.0).astype(np.float32)        # [sk, sq]
    mneg = np.full((128, 128), -3000.0, np.float32)           # lhsT, all cols

    xt8 = [np.ascontiguousarray(x[b].T).astype(FP8) for b in range(B)]
    xt16 = [np.ascontiguousarray(x[b].T).astype(BF16) for b in range(B)]

    in_maps = []
    for c in range(8):
        b, g = divmod(c, 4)
        wqk = np.concatenate(
            [Wq[:, g * FQ:(g + 1) * FQ] * SW,
             Wk[:, g * HD:(g + 1) * HD] * SW], axis=1).astype(FP8)
        wv = Wv[:, g * HD:(g + 1) * HD].astype(BF16)
        wo = (Wo[g * FQ:(g + 1) * FQ, :] * (1.0 / 16.0)).astype(BF16)
        in_maps.append({
            "xt8": xt8[b],
            "xt16": xt16[b],
            "wqk": wqk,
            "wv": wv,
            "wo": wo,
            "cos2": cos2,
            "sina2": sina2,
            "mt8": mt.astype(FP8),
            "mt16": mt.astype(BF16),
            "mneg": mneg.astype(BF16),
            "idb": np.eye(128, dtype=BF16),
            "idt16": np.eye(64, dtype=BF16),
        })
    return in_maps


def kernel(x, cos, sin, Wq, Wk, Wv, Wo):
    from concourse.bass_utils import run_bass_kernel_spmd

    nc = _build()
    in_maps = _prep_inputs(x, cos, sin, Wq, Wk, Wv, Wo)
    res = run_bass_kernel_spmd(nc, in_maps, list(range(8)))
    out = np.zeros((B, S, D), np.float32)
    for c in range(8):
        out[c // 4] += res.results[c]["out"].astype(np.float32)
    return out


# revision 26
# speedup vs baseline: 1.1925x; 1.1925x over previous
"""GQA attention (B=2, S=2048, D=1024, H=16, KV=4, HD=64) with RoPE + causal
softmax + output projection, sharded over 8 trn2 NeuronCores.

v2 design (per core: one batch x one KV group = 4 q heads + 1 kv head):
  - Q/K projections in fp8e4 DoubleRow (contract 1024 as 4 stacked pairs of
    128-row k-tiles, 2 cols/cycle).  Weights x64 on host; rope tables carry
    1/64 so qt/kt come out true-scale bf16.
  - V projection in bf16 (accuracy: early queries see V almost directly).
  - scores^T row-packed: two 64-contract matmuls on PE row halves 0:63/64:127
    run concurrently (kt/qt duplicated to partitions 64:128 via SBUF DMA).
  - exp on ACT with fused scale 1/sqrt(HD); one instr per chunk-pair, the
    above-diagonal strip is computed+exp'd then zeroed by the [0|tri] mask.
  - AV: span 0 (queries 0:512) in bf16; spans 1:4 in fp8e4 DoubleRow over
    chunk pairs (the 1/16 fp8 truncation bias is pre-compensated x1.03125;
    per-element fp8 noise averages out over >=512 keys).  The ones-columns
    in vaug give the softmax denominators for free.
  - out projection bf16; partial [S,D] outputs summed on host per batch.
  - interleaved schedule: QKV+rope span sp runs while attention span sp-1
    streams; elementwise work split across DVE and Pool.
"""

import sys

if "/opt/trn_rl_repo" not in sys.path:
    sys.path.insert(0, "/opt/trn_rl_repo")

import numpy as np
import ml_dtypes

B, S, D = 2, 2048, 1024
H, KV, HD = 16, 4, 64
NHC = H // KV          # query heads per core = 4
FQ = NHC * HD          # 256
FQK = FQ + HD          # 320 fp8 qk columns
SPAN = 512
NSPAN = S // SPAN      # 4
NCHUNK = S // 128      # 16
KD = D // 128          # 8
BF16 = ml_dtypes.bfloat16
FP8 = ml_dtypes.float8_e4m3

_CACHE = {}


def _build(debug_taps=False):
    key = ("nc", debug_taps)
    if key in _CACHE:
        return _CACHE[key]

    import concourse.bass as bass
    import concourse.tile as tile
    from concourse import bacc, mybir

    f32 = mybir.dt.float32
    bf16 = mybir.dt.bfloat16
    fp8 = mybir.dt.float8e4
    ADD = mybir.AluOpType.add
    MUL = mybir.AluOpType.mult
    EXP = mybir.ActivationFunctionType.Exp
    DR = mybir.MatmulPerfMode.DoubleRow
    ts = bass.ts

    nc = bacc.Bacc("TRN2", target_bir_lowering=False, debug=False)

    xt8_d = nc.dram_tensor("xt8", [D, S], fp8, kind="ExternalInput").ap()
    xt16_d = nc.dram_tensor("xt16", [D, S], bf16, kind="ExternalInput").ap()
    wqk_d = nc.dram_tensor("wqk", [D, FQK], fp8, kind="ExternalInput").ap()
    wv_d = nc.dram_tensor("wv", [D, HD], bf16, kind="ExternalInput").ap()
    wo_d = nc.dram_tensor("wo", [FQ, D], bf16, kind="ExternalInput").ap()
    cos2_d = nc.dram_tensor("cos2", [128, S], bf16, kind="ExternalInput").ap()
    sina2_d = nc.dram_tensor("sina2", [128, S], bf16, kind="ExternalInput").ap()
    mt8_d = nc.dram_tensor("mt8", [128, 128], fp8, kind="ExternalInput").ap()
    mt16_d = nc.dram_tensor("mt16", [128, 128], bf16, kind="ExternalInput").ap()
    mneg_d = nc.dram_tensor("mneg", [128, 128], bf16, kind="ExternalInput").ap()
    idb_d = nc.dram_tensor("idb", [128, 128], bf16, kind="ExternalInput").ap()
    idt16_d = nc.dram_tensor("idt16", [64, 64], bf16, kind="ExternalInput").ap()
    out_d = nc.dram_tensor("out", [S, D], bf16, kind="ExternalOutput").ap()

    if debug_taps:
        dbg_qt = nc.dram_tensor("dbg_qt", [128, NHC, S], bf16, kind="ExternalOutput").ap()
        dbg_kt = nc.dram_tensor("dbg_kt", [128, S], bf16, kind="ExternalOutput").ap()
        dbg_vaug8 = nc.dram_tensor("dbg_vaug8", [128, NCHUNK, 128], fp8, kind="ExternalOutput").ap()
        dbg_at = nc.dram_tensor("dbg_at", [128, 2, S], bf16, kind="ExternalOutput").ap()

    xt8_v = xt8_d.rearrange("(ko p) s -> p ko s", p=128)
    xt16_v = xt16_d.rearrange("(ko p) s -> p ko s", p=128)
    wqk_v = wqk_d.rearrange("(ko p) f -> p ko f", p=128)
    wv_v = wv_d.rearrange("(ko p) f -> p ko f", p=128)
    wo_v = wo_d.rearrange("(c p) n -> p c n", p=128)
    out_v = out_d.rearrange("(t p) n -> p t n", p=128)

    with tile.TileContext(nc) as tc:
        with tc.tile_pool(name="consts", bufs=1) as consts, \
             tc.tile_pool(name="persist", bufs=1) as persist, \
             tc.tile_pool(name="xtp", bufs=3) as xtp, \
             tc.tile_pool(name="ropet", bufs=4) as ropet, \
             tc.tile_pool(name="cpp", bufs=3) as cpp, \
             tc.tile_pool(name="vtp", bufs=2) as vtp, \
             tc.tile_pool(name="etp", bufs=6) as etp, \
             tc.tile_pool(name="lp", bufs=3) as lp, \
             tc.tile_pool(name="obp", bufs=4) as obp, \
             tc.tile_pool(name="psA", bufs=2, space="PSUM") as psA, \
             tc.tile_pool(name="psS", bufs=2, space="PSUM") as psS, \
             tc.tile_pool(name="psAV", bufs=1, space="PSUM") as psAV:

            wqk_sb = consts.tile([128, KD, FQK], fp8, name="wqk_sb")
            wv_sb = consts.tile([128, KD, HD], bf16, name="wv_sb")
            cos2_sb = consts.tile([128, S], bf16, name="cos2_sb")
            sina2_sb = consts.tile([128, S], bf16, name="sina2_sb")
            mt8_sb = consts.tile([128, 128], fp8, name="mt8_sb")
            mt16_sb = consts.tile([128, 128], bf16, name="mt16_sb")
            mneg_sb = consts.tile([128, 128], bf16, name="mneg_sb")
            idb_sb = consts.tile([128, 128], bf16, name="idb_sb")
            idt16_sb = consts.tile([64, 64], bf16, name="idt16_sb")
            wo_sb = consts.tile([128, 2, D], bf16, name="wo_sb")

            qt_sb = persist.tile([128, NHC, S], bf16, name="qt_sb")
            kt_sb = persist.tile([128, S], bf16, name="kt_sb")
            vaug8_sb = persist.tile([128, NCHUNK, 128], fp8, name="vaug8_sb")
            vaug16_sb = persist.tile([128, SPAN // 128, 128], bf16, name="vaug16_sb")
            at_sb = persist.tile([128, 2, S], bf16, name="at_sb")

            # minimal prefix for the first matmuls
            nc.sync.dma_start(wqk_sb[:], wqk_v[:])

            xts8_list, xts16_list = [], []
            for sp in range(NSPAN):
                rng = slice(sp * SPAN, (sp + 1) * SPAN)
                xts8 = xtp.tile([128, KD, SPAN], fp8, name="xts8", tag="x8")
                nc.sync.dma_start(xts8[:], xt8_v[:, :, rng])
                if sp == 0:
                    # rope tables + small mask/identity consts first: attn(0)
                    # diag pairs need them long before xts16 is consumed
                    nc.sync.dma_start(cos2_sb[:], cos2_d[:])
                    nc.sync.dma_start(sina2_sb[:], sina2_d[:])
                    nc.sync.dma_start(mneg_sb[:], mneg_d[:])
                    nc.sync.dma_start(idb_sb[:], idb_d[:])
                    nc.sync.dma_start(mt8_sb[:], mt8_d[:])
                    nc.sync.dma_start(mt16_sb[:], mt16_d[:])
                    nc.sync.dma_start(idt16_sb[:], idt16_d[:])
                    nc.sync.dma_start(wv_sb[:], wv_v[:])
                xts16 = xtp.tile([128, KD, SPAN], bf16, name="xts16", tag="x16")
                nc.sync.dma_start(xts16[:], xt16_v[:, :, rng])
                xts8_list.append(xts8)
                xts16_list.append(xts16)
                if sp == 0:
                    nc.gpsimd.memset(vaug8_sb[:, :, 0:64], 1.0)
                    nc.gpsimd.memset(vaug16_sb[:, :, 0:64], 1.0)
                if sp == 1:
                    nc.sync.dma_start(wo_sb[:], wo_v[:])

            def qkv_units(sp):
                """Closures: Q/K fp8-DR + rope per ftile, V proj, V transpose.
                Emitted interleaved between attention pairs so the PE stays
                dense and the ACT engine is fed continuously."""
                rng = slice(sp * SPAN, (sp + 1) * SPAN)
                xts8 = xts8_list[sp]
                xts16 = xts16_list[sp]
                vt16_cell = {}

                def ft_unit(ft):
                    def run():
                        fcols = 128 if ft < 2 else 64
                        ps = psA.tile([128, SPAN], f32, name="psqk", tag="pa")
                        for k in range(KD // 2):
                            nc.tensor.matmul(
                                ps[0:fcols, :],
                                lhsT=wqk_sb[:, 2 * k:2 * k + 2,
                                            ts(ft, 128) if ft < 2 else slice(FQ, FQK)],
                                rhs=xts8[:, 2 * k:2 * k + 2, :],
                                start=(k == 0),
                                stop=(k == KD // 2 - 1),
                                perf_mode=DR,
                            )
                        np_lo = fcols
                        # bounce psum to SBUF bf16 once; rope runs on DVE bf16
                        cp = cpp.tile([128, SPAN], bf16, name="cp", tag="cp")
                        nc.vector.tensor_copy(cp[0:np_lo, :], ps[0:np_lo])
                        t = ropet.tile([128, 2, SPAN], bf16, name="ropet", tag="rt")
                        nc.vector.tensor_tensor(
                            t[0:np_lo, 0, :], cp[0:np_lo], cos2_sb[0:np_lo, rng], op=MUL
                        )
                        for o in range(0, np_lo, 64):
                            nc.vector.tensor_tensor(
                                t[o:o + 32, 1, :], cp[o + 32:o + 64],
                                sina2_sb[o + 32:o + 64, rng], op=MUL,
                            )
                            nc.vector.tensor_tensor(
                                t[o + 32:o + 64, 1, :], cp[o:o + 32],
                                sina2_sb[o:o + 32, rng], op=MUL,
                            )
                        if ft < 2:
                            nc.vector.tensor_tensor(
                                qt_sb[0:64, 2 * ft, rng], t[0:64, 0, :], t[0:64, 1, :], op=ADD
                            )
                            nc.vector.tensor_tensor(
                                qt_sb[0:64, 2 * ft + 1, rng], t[64:128, 0, :], t[64:128, 1, :], op=ADD
                            )
                            # duplicate for row-packed scores as soon as ready
                            nc.sync.dma_start(
                                qt_sb[64:128, 2 * ft:2 * ft + 2, rng],
                                qt_sb[0:64, 2 * ft:2 * ft + 2, rng],
                            )
                        else:
                            nc.vector.tensor_tensor(
                                kt_sb[0:64, rng], t[0:64, 0, :], t[0:64, 1, :], op=ADD
                            )
                            nc.sync.dma_start(kt_sb[64:128, rng], kt_sb[0:64, rng])
                    return run

                def v_proj():
                    psv = psA.tile([128, SPAN], f32, name="psv", tag="pa")
                    for k in range(KD):
                        nc.tensor.matmul(
                            psv[0:64, :],
                            lhsT=wv_sb[:, k, :],
                            rhs=xts16[:, k, :],
                            start=(k == 0),
                            stop=(k == KD - 1),
                        )
                    vt16 = vtp.tile([64, SPAN], bf16, name="vt16", tag="vt16")
                    nc.vector.tensor_scalar_mul(vt16[:], psv[0:64, :], 16.0)
                    vt16_cell["vt"] = vt16

                def v_transp():
                    vt16 = vt16_cell["vt"]
                    tps16 = psA.tile([128, SPAN // 128, 64], bf16, name="tps16", tag="pa")
                    for c in range(SPAN // 128):
                        nc.tensor.transpose(tps16[:, c, :], vt16[:, ts(c, 128)], idt16_sb[:])
                    nc.vector.tensor_scalar_mul(
                        vaug8_sb[:, sp * 4:sp * 4 + 4, 64:128], tps16[:], 1.03125)
                    if sp == 0:
                        nc.vector.tensor_copy(vaug16_sb[:, 0:4, 64:128], tps16[:])

                return [ft_unit(2), ft_unit(0), ft_unit(1), v_proj, v_transp]

            def proj_units(s):
                """Out-projection closures for span s (at_sb already final)."""
                units = []
                for i, st in enumerate(range(4 * s, 4 * s + 4)):
                    for no in range(2):
                        def run(i=i, st=st, no=no):
                            pc = psA.tile([128, SPAN], f32, name="pc", tag="pa")
                            for c in range(2):
                                nc.tensor.matmul(
                                    pc[:],
                                    lhsT=at_sb[:, c, ts(st, 128)],
                                    rhs=wo_sb[:, c, ts(no, SPAN)],
                                    start=(c == 0),
                                    stop=(c == 1),
                                )
                            ob = obp.tile([128, SPAN], bf16, name="ob", tag="ob")
                            if (i + no) % 2 == 0:
                                nc.vector.tensor_copy(ob[:], pc[:])
                            else:
                                nc.scalar.copy(ob[:], pc[:])
                            nc.sync.dma_start(out_v[:, st, ts(no, SPAN)], ob[:])
                        units.append(run)
                return units

            def attn_emitters(s):
                """Pair closures (scores+exp+masks, AV pipelined one behind)
                and per-pp norm closures; returns the flat emission list."""
                npair = 2 * s + 2
                use8 = s > 0
                ctx = {"pending": None, "avs": None}

                def emit_av(hi, t, et):
                    av = ctx["avs"][hi]
                    j0, j1 = 2 * t, 2 * t + 1
                    lo0 = max(j0 * 128 - s * SPAN, 0)
                    lo1 = max(j1 * 128 - s * SPAN, 0)
                    if use8:
                        nc.tensor.matmul(
                            av[:, lo0:SPAN],
                            lhsT=vaug8_sb[:, j0:j0 + 2, :],
                            rhs=et[:, 0:2, lo0:SPAN],
                            start=(t == 0),
                            stop=(t == npair - 1),
                            perf_mode=DR,
                        )
                    else:
                        for par, j, lo in ((0, j0, lo0), (1, j1, lo1)):
                            nc.tensor.matmul(
                                av[:, lo:SPAN],
                                lhsT=vaug16_sb[:, j, :],
                                rhs=et[:, par, lo:SPAN],
                                start=(j == 0),
                                stop=(j == npair * 2 - 1),
                            )

                def pair(pp, hi, t):
                    def run():
                        if hi == 0 and t == 0:
                            ctx["avs"] = [
                                psAV.tile([128, SPAN], f32, name="av", tag="av", bufs=2)
                                for _ in range(2)]
                        h = 2 * pp + hi
                        j0, j1 = 2 * t, 2 * t + 1
                        lo0 = max(j0 * 128 - s * SPAN, 0)
                        diag = j0 >= 4 * s
                        sc = psS.tile([128, 2, SPAN], f32, name="sc", tag="sc")
                        # row-packed pair: PE rows 0:63 / 64:127; j1 from lo0,
                        # the invalid strip gets -3000 added so exp -> 0
                        nc.tensor.matmul(
                            sc[:, 0, lo0:SPAN],
                            lhsT=kt_sb[0:64, ts(j0, 128)],
                            rhs=qt_sb[0:64, h, s * SPAN + lo0:(s + 1) * SPAN],
                            start=True, stop=True,
                        )
                        nc.tensor.matmul(
                            sc[:, 1, lo0:SPAN],
                            lhsT=kt_sb[64:128, ts(j1, 128)],
                            rhs=qt_sb[64:128, h, s * SPAN + lo0:(s + 1) * SPAN],
                            start=True, stop=not diag,
                        )
                        if diag:
                            nc.tensor.matmul(
                                sc[:, 1, lo0:lo0 + 128],
                                lhsT=mneg_sb[:],
                                rhs=idb_sb[:],
                                start=False, stop=True,
                            )
                        if ctx["pending"] is not None:
                            emit_av(*ctx["pending"])
                            ctx["pending"] = None
                        et = etp.tile([128, 2, SPAN], fp8 if use8 else bf16,
                                      name="et", tag="et8" if use8 else "et16")
                        nc.scalar.activation(
                            et[:, :, lo0:SPAN], sc[:, :, lo0:SPAN], EXP,
                            scale=0.125,
                        )
                        if diag:
                            nc.gpsimd.tensor_tensor(
                                et[:, 0, lo0:lo0 + 128],
                                et[:, 0, lo0:lo0 + 128],
                                mt8_sb[:] if use8 else mt16_sb[:], op=MUL,
                            )
                            nc.gpsimd.tensor_tensor(
                                et[:, 1, lo0 + 128:lo0 + 256],
                                et[:, 1, lo0 + 128:lo0 + 256],
                                mt8_sb[:] if use8 else mt16_sb[:], op=MUL,
                            )
                        ctx["pending"] = (hi, t, et)
                    return run

                def norm_pp(pp):
                    def run():
                        if ctx["pending"] is not None:
                            emit_av(*ctx["pending"])
                            ctx["pending"] = None
                        for hi in range(2):
                            av = ctx["avs"][hi]
                            linv = lp.tile([64, SPAN], f32, name="linv", tag="lv")
                            nc.vector.reciprocal_approx_fast(
                                out=linv[:], in_=av[0:64, :]
                            )
                            dst = at_sb[64 * hi:64 * (hi + 1), pp,
                                        s * SPAN:(s + 1) * SPAN]
                            nc.vector.tensor_tensor(dst, av[64:128, :], linv[:], op=MUL)
                    return run

                items = []
                for pp in range(2):
                    for hi in range(2):
                        for t in range(npair):
                            items.append(pair(pp, hi, t))
                    items.append(norm_pp(pp))
                return items

            def run_interleaved(items, units):
                n, m = len(items), len(units)
                ui = 0
                for idx, it in enumerate(items):
                    it()
                    while ui < m and (idx + 1) * (m + 1) >= (ui + 1) * n:
                        units[ui]()
                        ui += 1
                while ui < m:
                    units[ui]()
                    ui += 1

            # two spans of QKV up front: the PE stays dense while both
            # ropes drain on DVE, so attention(0) starts stall-free
            for u in qkv_units(0):
                u()
            for u in qkv_units(1):
                u()
            run_interleaved(attn_emitters(0), qkv_units(2))
            run_interleaved(attn_emitters(1), qkv_units(3) + proj_units(0))
            run_interleaved(attn_emitters(2), proj_units(1))
            run_interleaved(attn_emitters(3), proj_units(2))
            for u in proj_units(3):
                u()

            if debug_taps:
                nc.sync.dma_start(dbg_qt[:], qt_sb[:])
                nc.sync.dma_start(dbg_kt[:], kt_sb[:])
                nc.sync.dma_start(dbg_vaug8[:], vaug8_sb[:])
                nc.sync.dma_start(dbg_at[:], at_sb[:])

    nc.compile()
    _CACHE[key] = nc
    return nc


def _prep_inputs(x, cos, sin, Wq, Wk, Wv, Wo):
    """Build the 8 per-core input maps (host-side sharding + layout prep)."""
    x = np.asarray(x, np.float32)
    cos = np.asarray(cos, np.float32)
    sin = np.asarray(sin, np.float32)
    Wq = np.asarray(Wq, np.float32)
    Wk = np.asarray(Wk, np.float32)
    Wv = np.asarray(Wv, np.float32)
    Wo = np.asarray(Wo, np.float32)

    SW = 64.0

    cosT = cos.T.copy() * (1.0 / SW)          # [HD, S]
    sinT = sin.T.copy() * (1.0 / SW)
    cos2 = np.tile(cosT, (2, 1)).astype(BF16)                 # [128, S]
    # indexed by the INPUT partition of the rotate-half read (SBUF+SBUF
    # tensor_tensor requires equal input base partitions)
    sina = np.concatenate([sinT[32:64], -sinT[0:32]], axis=0)
    sina2 = np.tile(sina, (2, 1)).astype(BF16)                # [128, S]

    p = np.arange(128)[:, None]
    f = np.arange(128)[None, :]
    mt = np.where(p <= f, 1.0, 0.0).astype(np.float32)        # [sk, sq]
    mneg = np.full((128, 128), -3000.0, np.float32)           # lhsT, all cols

    xt8 = [np.ascontiguousarray(x[b].T).astype(FP8) for b in range(B)]
    xt16 = [np.ascontiguousarray(x[b].T).astype(BF16) for b in range(B)]

    in_maps = []
    for c in range(8):
        b, g = divmod(c, 4)
        wqk = np.concatenate(
            [Wq[:, g * FQ:(g + 1) * FQ] * SW,
             Wk[:, g * HD:(g + 1) * HD] * SW], axis=1).astype(FP8)
        wv = Wv[:, g * HD:(g + 1) * HD].astype(BF16)
        wo = (Wo[g * FQ:(g + 1) * FQ, :] * (1.0 / 16.0)).astype(BF16)
        in_maps.append({
            "xt8": xt8[b],
            "xt16": xt16[b],
            "wqk": wqk,
            "wv": wv,
            "wo": wo,
            "cos2": cos2,
            "sina2": sina2,
            "mt8": mt.astype(FP8),
            "mt16": mt.astype(BF16),
            "mneg": mneg.astype(BF16),
            "idb": np.eye(128, dtype=BF16),
            "idt16": np.eye(64, dtype=BF16),
        })
    return in_maps


def kernel(x, cos, sin, Wq, Wk, Wv, Wo):
    from concourse.bass_utils import run_bass_kernel_spmd

    nc = _build()
    in_maps = _prep_inputs(x, cos, sin, Wq, Wk, Wv, Wo)
    res = run_bass_kernel_spmd(nc, in_maps, list(range(8)))
    out = np.zeros((B, S, D), np.float32)
    for c in range(8):
        out[c // 4] += res.results[c]["out"].astype(np.float32)
    return out
